# revision 8
# baseline (speedup 1.0000x reference)
"""MultiHeadAttention TRN2 kernel.

Problem: B=2, S=2048, D=1024, H=16, DK=64.
  qh/kh/vh = split_heads(x @ W);  scores = qh@khT / 8;  attn = softmax(scores)
  out = merge_heads(attn @ vh) @ w_o + b_o;   returns (out, attn)

Sharding: 8 cores; core c handles batch b=c//4, heads h0=(c%4)*4 .. h0+4.
Per-core inputs: q/k/v[b] (full), W_q/W_k/W_v column-slices for its 4 heads,
W_o row-slice.  Per-core outputs: attn slice [4,S,S] + partial out [S,D].
Host sums the 4 partials per batch and adds b_o.

Per-core pipeline (all matmuls bf16 with fp32 PSUM accumulation):
  phase 0: load + bf16-convert weights
  phase 1: per 128-row stripe: DMA q/k/v, bf16-convert (ACT), PE-transpose
           to get d-major blocks, then project to qhT/khT [64,S] and vh [S,64]
  phase 2 per head:
    pass A (natural [sq,sk]): QK matmul -> ACT exp(s/8) with accum_out row
           sums -> reciprocal -> DVE normalize -> DMA attn out
    pass B (transposed [sk,sq]): QK matmul -> ACT exp -> PV accumulation
           (pvT [64,sq]), normalized via partition-broadcast recip + DVE mul
  phase 3: out projection xT_h @ w_o_h summed over heads in PSUM -> partial out
"""

import numpy as np
from contextlib import ExitStack

B, S, D, H = 2, 2048, 1024, 16
DK = 64
HPC = 4  # heads per core
N_CORES = 8
P = 128
NT = S // P  # 16 stripe tiles
ND = D // P  # 8 d blocks
CH = 512     # free-dim chunk (one PSUM bank of fp32)
NC_CH = S // CH  # 4


def _emit(ctx, tc):
    import concourse.bass as bass
    import concourse.tile as tile
    from concourse import mybir
    from concourse.masks import make_identity

    nc = tc.nc
    f32 = mybir.dt.float32
    bf16 = mybir.dt.bfloat16
    Exp = mybir.ActivationFunctionType.Exp
    Copy = mybir.ActivationFunctionType.Copy
    PSUM = bass.MemorySpace.PSUM

    q_d = nc.dram_tensor("q", [S, D], f32, kind="ExternalInput").ap()
    k_d = nc.dram_tensor("k", [S, D], f32, kind="ExternalInput").ap()
    v_d = nc.dram_tensor("v", [S, D], f32, kind="ExternalInput").ap()
    wq_d = nc.dram_tensor("wq", [D, HPC * DK], f32, kind="ExternalInput").ap()
    wk_d = nc.dram_tensor("wk", [D, HPC * DK], f32, kind="ExternalInput").ap()
    wv_d = nc.dram_tensor("wv", [D, HPC * DK], f32, kind="ExternalInput").ap()
    wo_d = nc.dram_tensor("wo", [HPC * DK, D], f32, kind="ExternalInput").ap()
    attn_d = nc.dram_tensor("attn", [HPC, S, S], f32, kind="ExternalOutput").ap()
    outp_d = nc.dram_tensor("outp", [S, D], f32, kind="ExternalOutput").ap()

    consts = ctx.enter_context(tc.tile_pool(name="consts", bufs=1))
    ident_bf = consts.tile([P, P], bf16, tag="ident_bf")
    make_identity(nc, ident_bf)
    ident_f32 = consts.tile([P, P], f32, tag="ident_f32")
    make_identity(nc, ident_f32)

    persist = ctx.enter_context(tc.tile_pool(name="persist", bufs=1))

    def ptiles(base, n, shape, dt):
        return [persist.tile(shape, dt, name=f"{base}{i}", tag=f"{base}{i}")
                for i in range(n)]

    qhT = ptiles("qhT", HPC, [DK, S], bf16)
    khT = ptiles("khT", HPC, [DK, S], bf16)
    vh = ptiles("vh", HPC, [P, NT, DK], bf16)
    xT = ptiles("xT", HPC, [DK, S], bf16)
    wq_sb = ptiles("wq_sb", ND, [P, HPC * DK], bf16)
    wk_sb = ptiles("wk_sb", ND, [P, HPC * DK], bf16)
    wv_sb = ptiles("wv_sb", ND, [P, HPC * DK], bf16)
    wo_sb = ptiles("wo_sb", HPC, [DK, D], bf16)

    # ---- phase 0: weights ----
    with tc.tile_pool(name="wstage", bufs=2) as wst:
        for db in range(ND):
            for w_dram, w_tiles in ((wq_d, wq_sb), (wk_d, wk_sb), (wv_d, wv_sb)):
                stg = wst.tile([P, HPC * DK], f32, tag="w")
                nc.sync.dma_start(stg[:], w_dram[db * P:(db + 1) * P, :])
                nc.any.tensor_copy(w_tiles[db][:], stg[:])
        for h in range(HPC):
            stg = wst.tile([DK, D], f32, tag="wo")
            nc.sync.dma_start(stg[:], wo_d[h * DK:(h + 1) * DK, :])
            nc.any.tensor_copy(wo_sb[h][:], stg[:])

    # ---- phase 1: transpose + projections ----
    with tc.tile_pool(name="nat", bufs=3) as natp, \
         tc.tile_pool(name="tblk", bufs=2) as tbp, \
         tc.tile_pool(name="ps_t", bufs=2, space=PSUM) as ps_t, \
         tc.tile_pool(name="ps_p", bufs=2, space=PSUM) as ps_p:
        for which, src_d in (("q", q_d), ("k", k_d), ("v", v_d)):
            for st in range(NT):
                nat = natp.tile([P, D], f32, tag="nat")
                nc.sync.dma_start(nat[:], src_d[st * P:(st + 1) * P, :])
                natb = natp.tile([P, D], bf16, tag="natb")
                nc.scalar.activation(natb[:], nat[:], Copy)
                blks = tbp.tile([P, ND, P], bf16, tag="blks")
                for db in range(ND):
                    pst = ps_t.tile([P, P], bf16, tag="t")
                    nc.tensor.transpose(pst[:], natb[:, db * P:(db + 1) * P], ident_bf[:])
                    nc.any.tensor_copy(blks[:, db, :], pst[:])
                if which in ("q", "k"):
                    w_sb = wq_sb if which == "q" else wk_sb
                    dstT = qhT if which == "q" else khT
                    for pr in range(2):  # head pairs
                        acc = ps_p.tile([P, P], f32, tag="pacc")
                        for db in range(ND):
                            nc.tensor.matmul(
                                acc[:], w_sb[db][:, pr * P:(pr + 1) * P],
                                blks[:, db, :], start=(db == 0), stop=(db == ND - 1))
                        nc.any.tensor_copy(dstT[2 * pr][:, st * P:(st + 1) * P], acc[0:DK, :])
                        nc.any.tensor_copy(dstT[2 * pr + 1][:, st * P:(st + 1) * P], acc[DK:P, :])
                else:
                    acc = ps_p.tile([P, HPC * DK], f32, tag="vacc")
                    for db in range(ND):
                        nc.tensor.matmul(acc[:], blks[:, db, :], wv_sb[db][:],
                                         start=(db == 0), stop=(db == ND - 1))
                    for h in range(HPC):
                        nc.any.tensor_copy(vh[h][:, st, :], acc[:, h * DK:(h + 1) * DK])

    # ---- phase 2: attention per head ----
    with tc.tile_pool(name="ps_a", bufs=2, space=PSUM) as ps_a, \
         tc.tile_pool(name="ps_b", bufs=2, space=PSUM) as ps_b, \
         tc.tile_pool(name="ps_pv", bufs=2, space=PSUM) as ps_pv, \
         tc.tile_pool(name="ps_r", bufs=1, space=PSUM) as ps_r, \
         tc.tile_pool(name="expa", bufs=8) as expa_p, \
         tc.tile_pool(name="expt", bufs=3) as expt_p, \
         tc.tile_pool(name="sums", bufs=4) as sums_p, \
         tc.tile_pool(name="recips", bufs=2) as rap, \
         tc.tile_pool(name="recipt", bufs=2) as rtp, \
         tc.tile_pool(name="bcast", bufs=2) as bc_p:
        for h in range(HPC):
            recip_all = rap.tile([P, NT], f32, tag="ra")
            # pass A: natural orientation -> attn_weights + row-sum recips
            for t in range(NT):
                sums4 = sums_p.tile([P, NC_CH], f32, tag="s4")
                echunks = []
                for cc in range(NC_CH):
                    ps = ps_a.tile([P, CH], f32, tag="a")
                    nc.tensor.matmul(ps[:], qhT[h][:, t * P:(t + 1) * P],
                                     khT[h][:, cc * CH:(cc + 1) * CH],
                                     start=True, stop=True)
                    ech = expa_p.tile([P, CH], f32, tag="ea")
                    nc.scalar.activation(ech[:], ps[:], Exp, scale=0.125,
                                         accum_out=sums4[:, cc:cc + 1])
                    echunks.append(ech)
                sums = sums_p.tile([P, 1], f32, tag="s1")
                nc.vector.tensor_reduce(sums[:], sums4[:],
                                        axis=mybir.AxisListType.X,
                                        op=mybir.AluOpType.add)
                nc.vector.reciprocal(recip_all[:, t:t + 1], sums[:])
                for cc in range(NC_CH):
                    nc.vector.tensor_scalar_mul(echunks[cc][:], echunks[cc][:],
                                                recip_all[:, t:t + 1])
                    nc.sync.dma_start(attn_d[h, t * P:(t + 1) * P, cc * CH:(cc + 1) * CH],
                                      echunks[cc][:])
            # transpose recips to [NT, P], flatten to one row on partition 0,
            # then broadcast to all DK partitions for the pass-B normalize
            ps_rt = ps_r.tile([NT, P], f32, tag="rt")
            nc.tensor.transpose(ps_rt[:], recip_all[:], ident_f32[:])
            recipT = rtp.tile([NT, P], f32, tag="rT")
            nc.any.tensor_copy(recipT[:], ps_rt[:])
            recip_row = rtp.tile([1, S], f32, tag="rrow")
            nc.sync.dma_start(recip_row[:], recipT[:])
            bc = bc_p.tile([DK, S], f32, tag="bc")
            nc.gpsimd.partition_broadcast(bc[:], recip_row[0:1, :])
            # pass B: transposed orientation -> pvT = vh^T @ exp^T, normalize
            for cc in range(NC_CH):
                pv = ps_pv.tile([DK, CH], f32, tag="pv")
                prev = None
                for m in range(NT):
                    ps = ps_b.tile([P, CH], f32, tag="b")
                    nc.tensor.matmul(ps[:], khT[h][:, m * P:(m + 1) * P],
                                     qhT[h][:, cc * CH:(cc + 1) * CH],
                                     start=True, stop=True)
                    et = expt_p.tile([P, CH], bf16, tag="et")
                    nc.scalar.activation(et[:], ps[:], Exp, scale=0.125)
                    if prev is not None:
                        nc.tensor.matmul(pv[:], vh[h][:, m - 1, :], prev[:],
                                         start=(m == 1), stop=False)
                    prev = et
                nc.tensor.matmul(pv[:], vh[h][:, NT - 1, :], prev[:],
                                 start=False, stop=True)
                nc.vector.tensor_mul(xT[h][:, cc * CH:(cc + 1) * CH], pv[:],
                                     bc[:, cc * CH:(cc + 1) * CH])

    # ---- phase 3: output projection ----
    with tc.tile_pool(name="ps_o", bufs=2, space=PSUM) as ps_o, \
         tc.tile_pool(name="ostage", bufs=3) as osp:
        for t in range(NT):
            for oc in range(D // CH):
                acc = ps_o.tile([P, CH], f32, tag="o")
                for h in range(HPC):
                    nc.tensor.matmul(acc[:], xT[h][:, t * P:(t + 1) * P],
                                     wo_sb[h][:, oc * CH:(oc + 1) * CH],
                                     start=(h == 0), stop=(h == HPC - 1))
                ot = osp.tile([P, CH], f32, tag="ot")
                nc.any.tensor_copy(ot[:], acc[:])
                nc.sync.dma_start(outp_d[t * P:(t + 1) * P, oc * CH:(oc + 1) * CH], ot[:])


def build_nc():
    import concourse.tile as tile
    from concourse import bacc

    nc = bacc.Bacc("TRN2", target_bir_lowering=False, debug=False)
    with tile.TileContext(nc) as tc, ExitStack() as ctx:
        _emit(ctx, tc)
    nc.compile()
    return nc


_NC_CACHE = None
_RUN_KWARGS = {}
_LAST_RESULTS = None


def kernel(q, k, v, w_q, w_k, w_v, w_o, b_o):
    global _NC_CACHE
    from concourse.bass_utils import run_bass_kernel_spmd

    if _NC_CACHE is None:
        _NC_CACHE = build_nc()
    nc = _NC_CACHE

    q = np.asarray(q, dtype=np.float32)
    k = np.asarray(k, dtype=np.float32)
    v = np.asarray(v, dtype=np.float32)
    w_q = np.asarray(w_q, dtype=np.float32)
    w_k = np.asarray(w_k, dtype=np.float32)
    w_v = np.asarray(w_v, dtype=np.float32)
    w_o = np.asarray(w_o, dtype=np.float32)
    b_o = np.asarray(b_o, dtype=np.float32)

    in_maps = []
    for c in range(N_CORES):
        b = c // 4
        h0 = (c % 4) * HPC
        cols = slice(h0 * DK, (h0 + HPC) * DK)
        in_maps.append({
            "q": q[b], "k": k[b], "v": v[b],
            "wq": w_q[:, cols], "wk": w_k[:, cols], "wv": w_v[:, cols],
            "wo": w_o[cols, :],
        })

    br = run_bass_kernel_spmd(nc, in_maps, list(range(N_CORES)), **_RUN_KWARGS)
    globals()["_LAST_RESULTS"] = br
    globals()["_last_in_maps"] = in_maps
    res = br.results

    out = np.zeros((B, S, D), dtype=np.float32)
    attn = np.empty((B, H, S, S), dtype=np.float32)
    for c in range(N_CORES):
        b = c // 4
        h0 = (c % 4) * HPC
        attn[b, h0:h0 + HPC] = res[c]["attn"]
        out[b] += res[c]["outp"]
    out += b_o
    return out, attn


# revision 12
# speedup vs baseline: 1.2000x; 1.2000x over previous
"""MultiHeadAttention TRN2 kernel.

Problem: B=2, S=2048, D=1024, H=16, DK=64.
  qh/kh/vh = split_heads(x @ W);  scores = qh@khT / 8;  attn = softmax(scores)
  out = merge_heads(attn @ vh) @ w_o + b_o;   returns (out, attn)

Sharding: 8 cores; core c handles batch b=c//4, heads h0=(c%4)*4 .. h0+4.
Per-core inputs: q/k/v[b] (full), W_q/W_k/W_v column-slices for its 4 heads,
W_o row-slice.  Per-core outputs: attn slice [4,S,S] + partial out [S,D].
Host sums the 4 partials per batch and adds b_o.

Per-core pipeline (all matmuls bf16 with fp32 PSUM accumulation):
  phase 0: load + bf16-convert weights
  phase 1: per 128-row stripe: DMA q/k/v, bf16-convert (ACT), PE-transpose
           to get d-major blocks, then project to qhT/khT [64,S] and vh [S,64]
  phase 2 per head:
    pass A (natural [sq,sk]): QK matmul -> ACT exp(s/8) with accum_out row
           sums -> reciprocal -> DVE normalize -> DMA attn out
    pass B (transposed [sk,sq]): QK matmul -> ACT exp -> PV accumulation
           (pvT [64,sq]), normalized via partition-broadcast recip + DVE mul
  phase 3: out projection xT_h @ w_o_h summed over heads in PSUM -> partial out
"""

import numpy as np
from contextlib import ExitStack

B, S, D, H = 2, 2048, 1024, 16
DK = 64
HPC = 4  # heads per core
N_CORES = 8
P = 128
NT = S // P  # 16 stripe tiles
ND = D // P  # 8 d blocks
CH = 512     # free-dim chunk (one PSUM bank of fp32)
NC_CH = S // CH  # 4


def _emit(ctx, tc, stop_after=None):
    import concourse.bass as bass
    import concourse.tile as tile
    from concourse import mybir
    from concourse.masks import make_identity

    nc = tc.nc
    f32 = mybir.dt.float32
    bf16 = mybir.dt.bfloat16
    Exp = mybir.ActivationFunctionType.Exp
    Copy = mybir.ActivationFunctionType.Copy
    PSUM = bass.MemorySpace.PSUM

    q_d = nc.dram_tensor("q", [S, D], f32, kind="ExternalInput").ap()
    k_d = nc.dram_tensor("k", [S, D], f32, kind="ExternalInput").ap()
    v_d = nc.dram_tensor("v", [S, D], f32, kind="ExternalInput").ap()
    wq_d = nc.dram_tensor("wq", [D, HPC * DK], f32, kind="ExternalInput").ap()
    wk_d = nc.dram_tensor("wk", [D, HPC * DK], f32, kind="ExternalInput").ap()
    wv_d = nc.dram_tensor("wv", [D, HPC * DK], f32, kind="ExternalInput").ap()
    wo_d = nc.dram_tensor("wo", [HPC * DK, D], f32, kind="ExternalInput").ap()
    attn_d = nc.dram_tensor("attn", [HPC, S, S], f32, kind="ExternalOutput").ap()
    outp_d = nc.dram_tensor("outp", [S, D], f32, kind="ExternalOutput").ap()

    consts = ctx.enter_context(tc.tile_pool(name="consts", bufs=1))
    ident_bf = consts.tile([P, P], bf16, tag="ident_bf")
    make_identity(nc, ident_bf)
    ident_f32 = consts.tile([P, P], f32, tag="ident_f32")
    make_identity(nc, ident_f32)

    persist = ctx.enter_context(tc.tile_pool(name="persist", bufs=1))

    def ptiles(base, n, shape, dt):
        return [persist.tile(shape, dt, name=f"{base}{i}", tag=f"{base}{i}")
                for i in range(n)]

    qhT = ptiles("qhT", HPC, [DK, S], bf16)
    khT = ptiles("khT", HPC, [DK, S], bf16)
    vh = ptiles("vh", HPC, [P, NT, DK], bf16)
    xT = ptiles("xT", HPC, [DK, S], bf16)
    wq_sb = ptiles("wq_sb", ND, [P, HPC * DK], bf16)
    wk_sb = ptiles("wk_sb", ND, [P, HPC * DK], bf16)
    wv_sb = ptiles("wv_sb", ND, [P, HPC * DK], bf16)
    wo_sb = ptiles("wo_sb", HPC, [DK, D], bf16)

    # ---- phase 0: weights ----
    with tc.tile_pool(name="wstage", bufs=2) as wst:
        for db in range(ND):
            for w_dram, w_tiles in ((wq_d, wq_sb), (wk_d, wk_sb), (wv_d, wv_sb)):
                stg = wst.tile([P, HPC * DK], f32, tag="w")
                nc.sync.dma_start(stg[:], w_dram[db * P:(db + 1) * P, :])
                nc.any.tensor_copy(w_tiles[db][:], stg[:])
        for h in range(HPC):
            stg = wst.tile([DK, D], f32, tag="wo")
            nc.sync.dma_start(stg[:], wo_d[h * DK:(h + 1) * DK, :])
            nc.any.tensor_copy(wo_sb[h][:], stg[:])

    # ---- phase 1: transpose + projections ----
    with tc.tile_pool(name="nat", bufs=3) as natp, \
         tc.tile_pool(name="tblk", bufs=2) as tbp, \
         tc.tile_pool(name="ps_t", bufs=2, space=PSUM) as ps_t, \
         tc.tile_pool(name="ps_p", bufs=2, space=PSUM) as ps_p:
        for which, src_d in (("q", q_d), ("k", k_d), ("v", v_d)):
            for st in range(NT):
                nat = natp.tile([P, D], f32, tag="nat")
                nc.sync.dma_start(nat[:], src_d[st * P:(st + 1) * P, :])
                natb = natp.tile([P, D], bf16, tag="natb")
                nc.scalar.activation(natb[:], nat[:], Copy)
                blks = tbp.tile([P, ND, P], bf16, tag="blks")
                for db in range(ND):
                    pst = ps_t.tile([P, P], bf16, tag="t")
                    nc.tensor.transpose(pst[:], natb[:, db * P:(db + 1) * P], ident_bf[:])
                    nc.any.tensor_copy(blks[:, db, :], pst[:])
                if which in ("q", "k"):
                    w_sb = wq_sb if which == "q" else wk_sb
                    dstT = qhT if which == "q" else khT
                    for pr in range(2):  # head pairs
                        acc = ps_p.tile([P, P], f32, tag="pacc")
                        for db in range(ND):
                            nc.tensor.matmul(
                                acc[:], w_sb[db][:, pr * P:(pr + 1) * P],
                                blks[:, db, :], start=(db == 0), stop=(db == ND - 1))
                        nc.any.tensor_copy(dstT[2 * pr][:, st * P:(st + 1) * P], acc[0:DK, :])
                        nc.any.tensor_copy(dstT[2 * pr + 1][:, st * P:(st + 1) * P], acc[DK:P, :])
                else:
                    acc = ps_p.tile([P, HPC * DK], f32, tag="vacc")
                    for db in range(ND):
                        nc.tensor.matmul(acc[:], blks[:, db, :], wv_sb[db][:],
                                         start=(db == 0), stop=(db == ND - 1))
                    for h in range(HPC):
                        nc.any.tensor_copy(vh[h][:, st, :], acc[:, h * DK:(h + 1) * DK])

    if stop_after == "proj":
        return

    # ---- phase 2: attention per head ----
    with tc.tile_pool(name="ps_a", bufs=2, space=PSUM) as ps_a, \
         tc.tile_pool(name="ps_b", bufs=2, space=PSUM) as ps_b, \
         tc.tile_pool(name="ps_pv", bufs=2, space=PSUM) as ps_pv, \
         tc.tile_pool(name="ps_r", bufs=1, space=PSUM) as ps_r, \
         tc.tile_pool(name="expa", bufs=8) as expa_p, \
         tc.tile_pool(name="expt", bufs=3) as expt_p, \
         tc.tile_pool(name="sums", bufs=4) as sums_p, \
         tc.tile_pool(name="recips", bufs=2) as rap, \
         tc.tile_pool(name="recipt", bufs=2) as rtp, \
         tc.tile_pool(name="bcast", bufs=2) as bc_p:
        for h in range(HPC):
            recip_all = rap.tile([P, NT], f32, tag="ra")
            # pass A: natural orientation -> attn_weights + row-sum recips
            for t in range(NT):
                sums4 = sums_p.tile([P, NC_CH], f32, tag="s4")
                echunks = []
                for cc in range(NC_CH):
                    ps = ps_a.tile([P, CH], f32, tag="a")
                    nc.tensor.matmul(ps[:], qhT[h][:, t * P:(t + 1) * P],
                                     khT[h][:, cc * CH:(cc + 1) * CH],
                                     start=True, stop=True)
                    ech = expa_p.tile([P, CH], f32, tag="ea")
                    nc.scalar.activation(ech[:], ps[:], Exp, scale=0.125,
                                         accum_out=sums4[:, cc:cc + 1])
                    echunks.append(ech)
                sums = sums_p.tile([P, 1], f32, tag="s1")
                nc.vector.tensor_reduce(sums[:], sums4[:],
                                        axis=mybir.AxisListType.X,
                                        op=mybir.AluOpType.add)
                nc.vector.reciprocal(recip_all[:, t:t + 1], sums[:])
                for cc in range(NC_CH):
                    nc.vector.tensor_scalar_mul(echunks[cc][:], echunks[cc][:],
                                                recip_all[:, t:t + 1])
                    nc.sync.dma_start(attn_d[h, t * P:(t + 1) * P, cc * CH:(cc + 1) * CH],
                                      echunks[cc][:])
            # transpose recips to [NT, P], flatten to one row on partition 0,
            # then broadcast to all DK partitions for the pass-B normalize
            ps_rt = ps_r.tile([NT, P], f32, tag="rt")
            nc.tensor.transpose(ps_rt[:], recip_all[:], ident_f32[:])
            recipT = rtp.tile([NT, P], f32, tag="rT")
            nc.any.tensor_copy(recipT[:], ps_rt[:])
            recip_row = rtp.tile([1, S], f32, tag="rrow")
            nc.sync.dma_start(recip_row[:], recipT[:])
            bc = bc_p.tile([DK, S], f32, tag="bc")
            nc.gpsimd.partition_broadcast(bc[:], recip_row[0:1, :])
            # pass B: transposed orientation -> pvT = vh^T @ exp^T, normalize
            for cc in range(NC_CH):
                pv = ps_pv.tile([DK, CH], f32, tag="pv")
                prev = None
                for m in range(NT):
                    ps = ps_b.tile([P, CH], f32, tag="b")
                    nc.tensor.matmul(ps[:], khT[h][:, m * P:(m + 1) * P],
                                     qhT[h][:, cc * CH:(cc + 1) * CH],
                                     start=True, stop=True)
                    et = expt_p.tile([P, CH], bf16, tag="et")
                    nc.scalar.activation(et[:], ps[:], Exp, scale=0.125)
                    if prev is not None:
                        nc.tensor.matmul(pv[:], vh[h][:, m - 1, :], prev[:],
                                         start=(m == 1), stop=False)
                    prev = et
                nc.tensor.matmul(pv[:], vh[h][:, NT - 1, :], prev[:],
                                 start=False, stop=True)
                nc.vector.tensor_mul(xT[h][:, cc * CH:(cc + 1) * CH], pv[:],
                                     bc[:, cc * CH:(cc + 1) * CH])

    # ---- phase 3: output projection ----
    with tc.tile_pool(name="ps_o", bufs=2, space=PSUM) as ps_o, \
         tc.tile_pool(name="ostage", bufs=3) as osp:
        for t in range(NT):
            for oc in range(D // CH):
                acc = ps_o.tile([P, CH], f32, tag="o")
                for h in range(HPC):
                    nc.tensor.matmul(acc[:], xT[h][:, t * P:(t + 1) * P],
                                     wo_sb[h][:, oc * CH:(oc + 1) * CH],
                                     start=(h == 0), stop=(h == HPC - 1))
                ot = osp.tile([P, CH], f32, tag="ot")
                nc.any.tensor_copy(ot[:], acc[:])
                nc.sync.dma_start(outp_d[t * P:(t + 1) * P, oc * CH:(oc + 1) * CH], ot[:])


def build_nc():
    import concourse.tile as tile
    from concourse import bacc

    nc = bacc.Bacc("TRN2", target_bir_lowering=False, debug=False)
    with tile.TileContext(nc) as tc, ExitStack() as ctx:
        _emit(ctx, tc)
    nc.compile()
    return nc


_NC_CACHE = None
_RUN_KWARGS = {}
_LAST_RESULTS = None


def _make_in_maps(q, k, v, w_q, w_k, w_v, w_o):
    in_maps = []
    for c in range(N_CORES):
        b = c // 4
        h0 = (c % 4) * HPC
        cols = slice(h0 * DK, (h0 + HPC) * DK)
        in_maps.append({
            "q": q[b], "k": k[b], "v": v[b],
            "wq": w_q[:, cols], "wk": w_k[:, cols], "wv": w_v[:, cols],
            "wo": w_o[cols, :],
        })
    return in_maps


def kernel(q, k, v, w_q, w_k, w_v, w_o, b_o):
    global _NC_CACHE
    from concourse.bass_utils import run_bass_kernel_spmd

    if _NC_CACHE is None:
        _NC_CACHE = build_nc()
    nc = _NC_CACHE

    q = np.asarray(q, dtype=np.float32)
    k = np.asarray(k, dtype=np.float32)
    v = np.asarray(v, dtype=np.float32)
    w_q = np.asarray(w_q, dtype=np.float32)
    w_k = np.asarray(w_k, dtype=np.float32)
    w_v = np.asarray(w_v, dtype=np.float32)
    w_o = np.asarray(w_o, dtype=np.float32)
    b_o = np.asarray(b_o, dtype=np.float32)

    in_maps = _make_in_maps(q, k, v, w_q, w_k, w_v, w_o)

    br = run_bass_kernel_spmd(nc, in_maps, list(range(N_CORES)), **_RUN_KWARGS)
    globals()["_LAST_RESULTS"] = br
    globals()["_last_in_maps"] = in_maps
    res = br.results

    out = np.zeros((B, S, D), dtype=np.float32)
    attn = np.empty((B, H, S, S), dtype=np.float32)
    for c in range(N_CORES):
        b = c // 4
        h0 = (c % 4) * HPC
        attn[b, h0:h0 + HPC] = res[c]["attn"]
        out[b] += res[c]["outp"]
    out += b_o
    return out, attn
